# revision 6
# baseline (speedup 1.0000x reference)
"""Trainium2 Bass kernel for nn_BaselineAttention_25984552141259.

Problem: QKV [3, B=2, H=8, N=4096, d=64] fp32 ->
         out[b,h,n,:] = softmax(Q[b,h] @ K[b,h].T) @ V[b,h]

Sharding: B*H = 16 heads, embarrassingly parallel -> 2 heads per core on 8
NeuronCores. Host hands each core its Q^T/K^T pre-transposed (d on
partitions, zero-padded d=64->128 because K=64 matmuls never un-throttle the
PE HAM clock gate) plus V in natural [N, d] layout.

Precision plan (validated on host, end-to-end rel err ~4e-3 vs 2e-2 gate):
  Q^T, K^T in fp16, pre-scaled by 2^-5 / 2^-4 on host (fp16 keeps 11
  mantissa bits; the 2^-9 product scale is undone by the exp's scale=512).
  fp16 stationary K^T enables Fast Weight Load (halves LDWEIGHTS).
  P and V' in bf16 (P spans e^-85..e^22 -> needs bf16 range; 0.4% rounding
  on softmax weights averages out in the PV reduction).

Device algorithm per head (flash-attention style, S^T layout):
  S^T[m, n] = sum_d K^T[d, m] * Q^T[d, n]      (PE, fp16 x fp16)
  P^T = exp(512*S^T - 25)                       (split: most m-groups on ACT;
                                                the tail DVE_MGS groups run on
                                                DVE via two custom 8-stage ops:
                                                pass1 = cubic exp(v) poly
                                                squared, pass2 = 8 squarings
                                                -> exp(512 v). ACT is otherwise
                                                the co-bottleneck with PE.)
  O^T[d', n] = sum_m V'[m, d'] * P^T[m, n]      (PE, V' = [V | ones] bf16;
                                                row 64 = softmax denominator)
  copy O^T PSUM -> SBUF (DVE), DMA all 65 rows; host divides rows 0:64 by
  row 64 while unsharding (normalization off-device).
"""
import numpy as np
import ml_dtypes
from contextlib import ExitStack

import concourse.bass as bass
import concourse.tile as tile
from concourse import bacc, mybir
from concourse import dve_ops as _dvo
from concourse.bass_utils import run_bass_kernel_spmd
from concourse.dve_spec import (Spec, Src0, Src1, C0, C1, C2, C3, One, lower,
                                _spill_c3_to_src1)
from concourse.dve_uop import DveOpSpec

N_CORES = 8
B, H, N, D = 2, 8, 4096, 64
HEADS = B * H
HPC = HEADS // N_CORES          # heads per core = 2
NCHUNK = 512                    # n-tile (matmul moving free dim)
NCH = N // NCHUNK               # 8 n-chunks per head
MB = N // 128                   # 32 m-blocks of 128 keys
MGROUP = 2                      # m-blocks per exp group (2 PSUM banks)
KQUARTER = MB // 4              # m-blocks per K^T load piece
EXP_BIAS = -25.0
QSCALE = np.float32(2.0 ** -5)
KSCALE = np.float32(2.0 ** -4)  # product scale 2^-9; exp scale undoes it
DVE_MGS = frozenset({12, 13, 14, 15})   # m-groups whose exp runs on DVE

F32 = mybir.dt.float32
F32R = mybir.dt.float32r
F16 = mybir.dt.float16
BF16 = mybir.dt.bfloat16

_CACHE = {}


def _register_exp_ops():
    """Register the two custom DVE exp ops (in-process; the per-NEFF DVE
    table is generated from dve_ops.OPS at compile time).

    pass1: v = in0 + C1;  q = 1 + v(1 + v(C3 + v*C2));  out = q*q
           with C1 = -25/512, C2 = 1/6, C3 = 0.5 (via in1 spill)
           => out = exp(2v) * (1 + O(v^4/24))
    pass2: out = in0^256  (8 squarings)
    Chain: exp(512*in - 25), rel err < ~7e-4 over the relevant score range.
    """
    if hasattr(_dvo, "ANT_EXP_P1"):
        return _dvo.ANT_EXP_P1, _dvo.ANT_EXP_P2

    v = Src0 + C1
    q = One + v * (One + v * (C3 + v * C2))
    spec1 = Spec(
        body=_spill_c3_to_src1(q * q),
        reference=lambda in0, in1, s0, s1, imm2: (
            lambda vv: (1 + vv * (1 + vv * (np.float32(in1) + vv * np.float32(imm2)))) ** 2
        )(np.asarray(in0, np.float32) + np.float32(s1)),
    )
    x = Src0
    for _ in range(8):
        x = x * x
    spec2 = Spec(body=x)

    ops = []
    for name, spec, rd1 in (("ANT_EXP_P1", spec1, True),
                            ("ANT_EXP_P2", spec2, False)):
        row = _dvo._CUSTOM_DVE_ROW_BASE + len(_dvo.OPS)
        assert row < 0x20, "custom DVE opcode rows exhausted"
        shas = {}
        for ver in ("v3", "v4"):
            u = lower(spec, ver=ver)
            shas[ver] = DveOpSpec(name=name, opcode=row, uops=u,
                                  rd1_en=rd1).sha(ver)
        op = _dvo.DveOp(name, spec, subdim=False, uops_sha=shas)
        _dvo.OPS.append(op)
        _dvo.CUSTOM_DVE_SPECS[name] = spec
        _dvo._SUB_OPCODE_FOR_NAME[name] = row
        setattr(_dvo, name, op)
        ops.append(op)
    return ops


def _build():
    exp_p1, exp_p2 = _register_exp_ops()
    nc = bacc.Bacc("TRN2", target_bir_lowering=False, debug=False,
                   num_devices=N_CORES)
    qt_d = nc.dram_tensor("qt", [HPC, 128, N], F16, kind="ExternalInput").ap()
    kt_d = nc.dram_tensor("kt", [HPC, 128, N], F16, kind="ExternalInput").ap()
    v_d = nc.dram_tensor("v", [HPC, N, D], BF16, kind="ExternalInput").ap()
    ot_d = nc.dram_tensor("ot", [HPC, D + 1, N], F32, kind="ExternalOutput").ap()

    with tile.TileContext(nc) as tc, ExitStack() as ctx:
        const = ctx.enter_context(tc.tile_pool(name="const", bufs=1))
        qk = ctx.enter_context(tc.tile_pool(name="qk", bufs=2))
        vpool = ctx.enter_context(tc.tile_pool(name="vpool", bufs=2))
        pexp = ctx.enter_context(tc.tile_pool(name="pexp", bufs=8))
        tmpp = ctx.enter_context(tc.tile_pool(name="tmpp", bufs=3))
        opool = ctx.enter_context(tc.tile_pool(name="opool", bufs=3))
        s_ps = ctx.enter_context(tc.tile_pool(name="s_ps", bufs=3, space="PSUM"))
        ot_ps = ctx.enter_context(tc.tile_pool(name="ot_ps", bufs=2, space="PSUM"))

        bias_t = const.tile([128, 1], F32)
        nc.vector.memset(bias_t[:], EXP_BIAS)
        half_t = const.tile([128, 1], F32)
        nc.vector.memset(half_t[:], 0.5)

        kt_all, qt_all, v_all = [], [], []
        for h in range(HPC):
            with nc.named_scope(f"load{h}"):
                # split loads so the first m-blocks/chunks arrive (and
                # compute starts) before the rest of the head lands
                kt_s = []
                qt_s = []
                v_s = []
                v_re = v_d[h].rearrange("(t p) d -> p t d", p=128)
                for i in range(4):
                    kq = qk.tile([128, KQUARTER, 128], F16, tag=f"kt{i}",
                                 name=f"kt_{h}_{i}")
                    nc.gpsimd.dma_start(
                        kq[:],
                        kt_d[h, :, bass.ts(i, KQUARTER * 128)].rearrange(
                            "p (t q) -> p t q", q=128),
                    )
                    kt_s.append(kq)
                    qq = qk.tile([128, 2, NCHUNK], F16, tag=f"qt{i}",
                                 name=f"qt_{h}_{i}")
                    nc.gpsimd.dma_start(
                        qq[:],
                        qt_d[h, :, bass.ts(i, 2 * NCHUNK)].rearrange(
                            "p (t q) -> p t q", q=NCHUNK),
                    )
                    qt_s.append(qq)
                    # V' piece [m-part, m-tile, d+1]; col 64 = 1.0 (row sums)
                    vq = vpool.tile([128, KQUARTER, D + 1], BF16, tag=f"v{i}",
                                    name=f"v_{h}_{i}")
                    nc.gpsimd.dma_start(
                        vq[:, :, 0:D],
                        v_re[:, bass.ts(i, KQUARTER), :],
                    )
                    nc.vector.memset(vq[:, :, D], 1.0)
                    v_s.append(vq)
                kt_all.append(kt_s)
                qt_all.append(qt_s)
                v_all.append(v_s)

        for h in range(HPC):
            kt_s, qt_s, v_s = kt_all[h], qt_all[h], v_all[h]
            with nc.named_scope(f"head{h}"):
                for nch in range(NCH):
                    n_sl = bass.ts(nch, NCHUNK)
                    qt_c = qt_s[nch // 2][:, nch % 2, :]
                    ot_t = ot_ps.tile([D + 1, NCHUNK], F32, tag="ot",
                                      name=f"ot_{h}_{nch}")
                    for mg in range(MB // MGROUP):
                        s_t = s_ps.tile([128, MGROUP, NCHUNK], F32, tag="s")
                        for j in range(MGROUP):
                            m = mg * MGROUP + j
                            nc.tensor.matmul(
                                s_t[:, j, :],
                                kt_s[m // KQUARTER][:, m % KQUARTER, :],
                                qt_c,
                                start=True, stop=True,
                            )
                        p_t = pexp.tile([128, MGROUP, NCHUNK], BF16, tag="p")
                        if mg in DVE_MGS:
                            t_t = tmpp.tile([128, MGROUP, NCHUNK], F32,
                                            tag="exptmp")
                            nc.vector._custom_dve(
                                exp_p1, out=t_t[:], in0=s_t[:],
                                in1=half_t[:],
                                s1=float(EXP_BIAS / 512.0),
                                imm2=float(1.0 / 6.0),
                            )
                            nc.vector._custom_dve(
                                exp_p2, out=p_t[:], in0=t_t[:],
                            )
                        else:
                            nc.scalar.activation(
                                p_t[:], s_t[:],
                                mybir.ActivationFunctionType.Exp,
                                bias=bias_t[:], scale=512.0,
                            )
                        for j in range(MGROUP):
                            m = mg * MGROUP + j
                            nc.tensor.matmul(
                                ot_t[:],
                                v_s[m // KQUARTER][:, m % KQUARTER, :],
                                p_t[:, j, :],
                                start=(m == 0), stop=(m == MB - 1),
                            )
                    # PSUM -> SBUF, then ship all 65 rows; host normalizes.
                    o_t = opool.tile([D + 1, NCHUNK], F32, tag="o")
                    nc.vector.tensor_copy(o_t[:], ot_t[:])
                    nc.sync.dma_start(ot_d[h][:, n_sl], o_t[:])

    nc.compile()
    return nc


def _get_nc():
    if "nc" not in _CACHE:
        _CACHE["nc"] = _build()
    return _CACHE["nc"]


def _make_in_maps(QKV):
    QKV = np.asarray(QKV, dtype=np.float32)
    q = QKV[0].reshape(HEADS, N, D)
    k = QKV[1].reshape(HEADS, N, D)
    v = QKV[2].reshape(HEADS, N, D)
    # zero-pad the contraction dim to 128: K=64 matmuls never un-throttle
    # the PE HAM clock gate (measured); K=128 runs at 2.4 GHz.
    qt = np.zeros((HEADS, 128, N), np.float16)
    qt[:, :D] = (q.transpose(0, 2, 1) * QSCALE).astype(np.float16)
    kt = np.zeros((HEADS, 128, N), np.float16)
    kt[:, :D] = (k.transpose(0, 2, 1) * KSCALE).astype(np.float16)
    v16 = v.astype(ml_dtypes.bfloat16)
    in_maps = []
    for c in range(N_CORES):
        sl = slice(c * HPC, (c + 1) * HPC)
        in_maps.append({
            "qt": qt[sl],
            "kt": kt[sl],
            "v": np.ascontiguousarray(v16[sl]),
        })
    return in_maps


def _assemble(results):
    ot = np.stack([r["ot"] for r in results])            # [8, 2, 65, 4096]
    ot = ot.reshape(HEADS, D + 1, N)
    out = ot[:, 0:D, :] / ot[:, D:D + 1, :]              # normalize on host
    out = out.transpose(0, 2, 1)                         # [16, 4096, 64]
    return np.ascontiguousarray(out).reshape(B, H, N, D).astype(np.float32)


def kernel(QKV):
    nc = _get_nc()
    res = run_bass_kernel_spmd(nc, _make_in_maps(QKV), list(range(N_CORES)))
    return _assemble(res.results)


# revision 7
# speedup vs baseline: 1.2479x; 1.2479x over previous
"""Trainium2 Bass kernel for nn_BaselineAttention_25984552141259.

Problem: QKV [3, B=2, H=8, N=4096, d=64] fp32 ->
         out[b,h,n,:] = softmax(Q[b,h] @ K[b,h].T) @ V[b,h]

Sharding: B*H = 16 heads, embarrassingly parallel -> 2 heads per core on 8
NeuronCores. Host hands each core its Q^T/K^T pre-transposed (d on
partitions, zero-padded d=64->128 because K=64 matmuls never un-throttle the
PE HAM clock gate) plus V in natural [N, d] layout.

Precision plan (validated on host, end-to-end rel err ~4e-3 vs 2e-2 gate):
  Q^T, K^T in fp16, pre-scaled by 2^-5 / 2^-4 on host (fp16 keeps 11
  mantissa bits; the 2^-9 product scale is undone by the exp's scale=512).
  fp16 stationary K^T enables Fast Weight Load (halves LDWEIGHTS).
  P and V' in bf16 (P spans e^-85..e^22 -> needs bf16 range; 0.4% rounding
  on softmax weights averages out in the PV reduction).

Device algorithm per head (flash-attention style, S^T layout):
  S^T[m, n] = sum_d K^T[d, m] * Q^T[d, n]      (PE, fp16 x fp16)
  P^T = exp(512*S^T - 25)                       (split: most m-groups on ACT;
                                                the tail DVE_MGS groups run on
                                                DVE via two custom 8-stage ops:
                                                pass1 = cubic exp(v) poly
                                                squared, pass2 = 8 squarings
                                                -> exp(512 v). ACT is otherwise
                                                the co-bottleneck with PE.)
  O^T[d', n] = sum_m V'[m, d'] * P^T[m, n]      (PE, V' = [V | ones] bf16;
                                                row 64 = softmax denominator)
  copy O^T PSUM -> SBUF (DVE), DMA all 65 rows; host divides rows 0:64 by
  row 64 while unsharding (normalization off-device).
"""
import numpy as np
import ml_dtypes
from contextlib import ExitStack

import concourse.bass as bass
import concourse.tile as tile
from concourse import bacc, mybir
from concourse import dve_ops as _dvo
from concourse.bass_utils import run_bass_kernel_spmd
from concourse.dve_spec import (Spec, Src0, Src1, C0, C1, C2, C3, One, lower,
                                _spill_c3_to_src1)
from concourse.dve_uop import DveOpSpec

N_CORES = 8
B, H, N, D = 2, 8, 4096, 64
HEADS = B * H
HPC = HEADS // N_CORES          # heads per core = 2
NCHUNK = 512                    # n-tile (matmul moving free dim)
NCH = N // NCHUNK               # 8 n-chunks per head
MB = N // 128                   # 32 m-blocks of 128 keys
MGROUP = 2                      # m-blocks per exp group (2 PSUM banks)
KQUARTER = MB // 4              # m-blocks per K^T load piece
EXP_BIAS = -25.0
QSCALE = np.float32(2.0 ** -5)
KSCALE = np.float32(2.0 ** -4)  # product scale 2^-9; exp scale undoes it
DVE_MGS = frozenset({2, 5, 8, 11, 14})  # m-groups whose exp runs on DVE
                                        # (spread: a clustered tail serializes
                                        # on the DVE queue and stalls the PE
                                        # via s_ps slot starvation)

F32 = mybir.dt.float32
F32R = mybir.dt.float32r
F16 = mybir.dt.float16
BF16 = mybir.dt.bfloat16

_CACHE = {}


def _register_exp_ops():
    """Register the two custom DVE exp ops (in-process; the per-NEFF DVE
    table is generated from dve_ops.OPS at compile time).

    pass1: v = in0 + C1;  q = 1 + v(1 + v(C3 + v*C2));  out = q*q
           with C1 = -25/512, C2 = 1/6, C3 = 0.5 (via in1 spill)
           => out = exp(2v) * (1 + O(v^4/24))
    pass2: out = in0^256  (8 squarings)
    Chain: exp(512*in - 25), rel err < ~7e-4 over the relevant score range.
    """
    if hasattr(_dvo, "ANT_EXP_P1"):
        return _dvo.ANT_EXP_P1, _dvo.ANT_EXP_P2

    v = Src0 + C1
    q = One + v * (One + v * (C3 + v * C2))
    spec1 = Spec(
        body=_spill_c3_to_src1(q * q),
        reference=lambda in0, in1, s0, s1, imm2: (
            lambda vv: (1 + vv * (1 + vv * (np.float32(in1) + vv * np.float32(imm2)))) ** 2
        )(np.asarray(in0, np.float32) + np.float32(s1)),
    )
    x = Src0
    for _ in range(8):
        x = x * x
    spec2 = Spec(body=x)

    ops = []
    for name, spec, rd1 in (("ANT_EXP_P1", spec1, True),
                            ("ANT_EXP_P2", spec2, False)):
        row = _dvo._CUSTOM_DVE_ROW_BASE + len(_dvo.OPS)
        assert row < 0x20, "custom DVE opcode rows exhausted"
        shas = {}
        for ver in ("v3", "v4"):
            u = lower(spec, ver=ver)
            shas[ver] = DveOpSpec(name=name, opcode=row, uops=u,
                                  rd1_en=rd1).sha(ver)
        op = _dvo.DveOp(name, spec, subdim=False, uops_sha=shas)
        _dvo.OPS.append(op)
        _dvo.CUSTOM_DVE_SPECS[name] = spec
        _dvo._SUB_OPCODE_FOR_NAME[name] = row
        setattr(_dvo, name, op)
        ops.append(op)
    return ops


def _build():
    exp_p1, exp_p2 = _register_exp_ops()
    nc = bacc.Bacc("TRN2", target_bir_lowering=False, debug=False,
                   num_devices=N_CORES)
    qt_d = nc.dram_tensor("qt", [HPC, 128, N], F16, kind="ExternalInput").ap()
    kt_d = nc.dram_tensor("kt", [HPC, 128, N], F16, kind="ExternalInput").ap()
    v_d = nc.dram_tensor("v", [HPC, N, D], BF16, kind="ExternalInput").ap()
    ot_d = nc.dram_tensor("ot", [HPC, D + 1, N], F32, kind="ExternalOutput").ap()

    with tile.TileContext(nc) as tc, ExitStack() as ctx:
        const = ctx.enter_context(tc.tile_pool(name="const", bufs=1))
        qk = ctx.enter_context(tc.tile_pool(name="qk", bufs=2))
        vpool = ctx.enter_context(tc.tile_pool(name="vpool", bufs=2))
        pexp = ctx.enter_context(tc.tile_pool(name="pexp", bufs=8))
        tmpp = ctx.enter_context(tc.tile_pool(name="tmpp", bufs=3))
        opool = ctx.enter_context(tc.tile_pool(name="opool", bufs=3))
        s_ps = ctx.enter_context(tc.tile_pool(name="s_ps", bufs=3, space="PSUM"))
        ot_ps = ctx.enter_context(tc.tile_pool(name="ot_ps", bufs=2, space="PSUM"))

        bias_t = const.tile([128, 1], F32)
        nc.vector.memset(bias_t[:], EXP_BIAS)
        half_t = const.tile([128, 1], F32)
        nc.vector.memset(half_t[:], 0.5)

        kt_all, qt_all, v_all = [], [], []
        for h in range(HPC):
            with nc.named_scope(f"load{h}"):
                # split loads so the first m-blocks/chunks arrive (and
                # compute starts) before the rest of the head lands
                kt_s = []
                qt_s = []
                v_s = []
                v_re = v_d[h].rearrange("(t p) d -> p t d", p=128)
                for i in range(4):
                    kq = qk.tile([128, KQUARTER, 128], F16, tag=f"kt{i}",
                                 name=f"kt_{h}_{i}")
                    nc.gpsimd.dma_start(
                        kq[:],
                        kt_d[h, :, bass.ts(i, KQUARTER * 128)].rearrange(
                            "p (t q) -> p t q", q=128),
                    )
                    kt_s.append(kq)
                    qq = qk.tile([128, 2, NCHUNK], F16, tag=f"qt{i}",
                                 name=f"qt_{h}_{i}")
                    nc.gpsimd.dma_start(
                        qq[:],
                        qt_d[h, :, bass.ts(i, 2 * NCHUNK)].rearrange(
                            "p (t q) -> p t q", q=NCHUNK),
                    )
                    qt_s.append(qq)
                    # V' piece [m-part, m-tile, d+1]; col 64 = 1.0 (row sums)
                    vq = vpool.tile([128, KQUARTER, D + 1], BF16, tag=f"v{i}",
                                    name=f"v_{h}_{i}")
                    nc.gpsimd.dma_start(
                        vq[:, :, 0:D],
                        v_re[:, bass.ts(i, KQUARTER), :],
                    )
                    nc.vector.memset(vq[:, :, D], 1.0)
                    v_s.append(vq)
                kt_all.append(kt_s)
                qt_all.append(qt_s)
                v_all.append(v_s)

        for h in range(HPC):
            kt_s, qt_s, v_s = kt_all[h], qt_all[h], v_all[h]
            with nc.named_scope(f"head{h}"):
                for nch in range(NCH):
                    n_sl = bass.ts(nch, NCHUNK)
                    qt_c = qt_s[nch // 2][:, nch % 2, :]
                    ot_t = ot_ps.tile([D + 1, NCHUNK], F32, tag="ot",
                                      name=f"ot_{h}_{nch}")
                    for mg in range(MB // MGROUP):
                        s_t = s_ps.tile([128, MGROUP, NCHUNK], F32, tag="s")
                        for j in range(MGROUP):
                            m = mg * MGROUP + j
                            nc.tensor.matmul(
                                s_t[:, j, :],
                                kt_s[m // KQUARTER][:, m % KQUARTER, :],
                                qt_c,
                                start=True, stop=True,
                            )
                        p_t = pexp.tile([128, MGROUP, NCHUNK], BF16, tag="p")
                        if mg in DVE_MGS:
                            t_t = tmpp.tile([128, MGROUP, NCHUNK], F32,
                                            tag="exptmp")
                            nc.vector._custom_dve(
                                exp_p1, out=t_t[:], in0=s_t[:],
                                in1=half_t[:],
                                s1=float(EXP_BIAS / 512.0),
                                imm2=float(1.0 / 6.0),
                            )
                            nc.vector._custom_dve(
                                exp_p2, out=p_t[:], in0=t_t[:],
                            )
                        else:
                            nc.scalar.activation(
                                p_t[:], s_t[:],
                                mybir.ActivationFunctionType.Exp,
                                bias=bias_t[:], scale=512.0,
                            )
                        for j in range(MGROUP):
                            m = mg * MGROUP + j
                            nc.tensor.matmul(
                                ot_t[:],
                                v_s[m // KQUARTER][:, m % KQUARTER, :],
                                p_t[:, j, :],
                                start=(m == 0), stop=(m == MB - 1),
                            )
                    # PSUM -> SBUF, then ship all 65 rows; host normalizes.
                    o_t = opool.tile([D + 1, NCHUNK], F32, tag="o")
                    nc.vector.tensor_copy(o_t[:], ot_t[:])
                    nc.sync.dma_start(ot_d[h][:, n_sl], o_t[:])

    nc.compile()
    return nc


def _get_nc():
    if "nc" not in _CACHE:
        _CACHE["nc"] = _build()
    return _CACHE["nc"]


def _make_in_maps(QKV):
    QKV = np.asarray(QKV, dtype=np.float32)
    q = QKV[0].reshape(HEADS, N, D)
    k = QKV[1].reshape(HEADS, N, D)
    v = QKV[2].reshape(HEADS, N, D)
    # zero-pad the contraction dim to 128: K=64 matmuls never un-throttle
    # the PE HAM clock gate (measured); K=128 runs at 2.4 GHz.
    qt = np.zeros((HEADS, 128, N), np.float16)
    qt[:, :D] = (q.transpose(0, 2, 1) * QSCALE).astype(np.float16)
    kt = np.zeros((HEADS, 128, N), np.float16)
    kt[:, :D] = (k.transpose(0, 2, 1) * KSCALE).astype(np.float16)
    v16 = v.astype(ml_dtypes.bfloat16)
    in_maps = []
    for c in range(N_CORES):
        sl = slice(c * HPC, (c + 1) * HPC)
        in_maps.append({
            "qt": qt[sl],
            "kt": kt[sl],
            "v": np.ascontiguousarray(v16[sl]),
        })
    return in_maps


def _assemble(results):
    ot = np.stack([r["ot"] for r in results])            # [8, 2, 65, 4096]
    ot = ot.reshape(HEADS, D + 1, N)
    out = ot[:, 0:D, :] / ot[:, D:D + 1, :]              # normalize on host
    out = out.transpose(0, 2, 1)                         # [16, 4096, 64]
    return np.ascontiguousarray(out).reshape(B, H, N, D).astype(np.float32)


def kernel(QKV):
    nc = _get_nc()
    res = run_bass_kernel_spmd(nc, _make_in_maps(QKV), list(range(N_CORES)))
    return _assemble(res.results)
